# revision 14
# baseline (speedup 1.0000x reference)
"""Data-parallel attention kernel for Trainium2 (8 NeuronCores).

Reference computation (per batch item b):
    scores[q, k] = sum_{hw} query[b, hw, q] * keys[b, hw, k]     (C=256, HW=4096)
    attn = softmax_k(scores)
    out[b, q, hw] = sum_k attn[q, k] * values[b, hw, k]

Sharding: batch axis (B=32) split across 8 cores, 4 items per core, no
cross-core communication.

Per-core per-item plan:
  S phase:  fp32 matmuls, contraction over hw streamed in 4 groups of 1024
            rows, accumulating into PSUM ([128, 2, 256] = 2 q-blocks).
  softmax:  DVE row-max (negated) -> ACT exp(in + bias) with accumulated row
            sums -> DVE reciprocal. Normalization is folded into the O-phase
            PSUM->SBUF epilogue, so A stays unnormalized bf16.
  O phase:  V streamed in 8 groups of 512 rows, cast to bf16 (ACT), PE
            transposed ([hw,k] -> [k,hw]) via identity matmuls, then bf16
            matmuls A @ V^T accumulated over the 2 k-chunks; epilogue scales
            rows by 1/rowsum during the PSUM->SBUF copy and DMAs out.
"""

import numpy as np

import concourse.bass as bass
import concourse.tile as tile
from concourse import bacc, mybir
from concourse.bass_utils import run_bass_kernel_spmd
from concourse.masks import make_identity
from contextlib import ExitStack

B, H, W, C = 32, 64, 64, 256
N_CORES = 8
B_LOC = B // N_CORES          # 4 batch items per core
HW = H * W                    # 4096
P = 128                       # partitions
N_CHUNK = HW // P             # 32 chunks of 128 hw-rows
SG = 8                        # chunks per S-phase group (1024 hw rows)
VG = 4                        # chunks per O-phase group (512 hw rows)
N_SGRP = N_CHUNK // SG        # 4
N_VGRP = N_CHUNK // VG        # 8
QB = C // P                   # 2 q-blocks
KC = C // P                   # 2 k-chunks

F32 = mybir.dt.float32
F32R = mybir.dt.float32r
BF16 = mybir.dt.bfloat16

_CACHE = {}


def _build():
    nc = bacc.Bacc("TRN2", target_bir_lowering=False, debug=False,
                   num_devices=N_CORES)
    q_ext = nc.dram_tensor("query", [B_LOC, H, W, C], F32,
                           kind="ExternalInput").ap()
    k_ext = nc.dram_tensor("keys", [B_LOC, H, W, C], F32,
                           kind="ExternalInput").ap()
    v_ext = nc.dram_tensor("values", [B_LOC, H, W, C], F32,
                           kind="ExternalInput").ap()
    o_ext = nc.dram_tensor("out", [B_LOC, C, H, W], F32,
                           kind="ExternalOutput").ap()

    # [b, hw, c] -> [b, p, n, c] where hw = n*128 + p
    qv = q_ext.rearrange("b h w c -> b (h w) c").rearrange(
        "b (n p) c -> b p n c", p=P)
    kv = k_ext.rearrange("b h w c -> b (h w) c").rearrange(
        "b (n p) c -> b p n c", p=P)
    vv = v_ext.rearrange("b h w c -> b (h w) c").rearrange(
        "b (n p) c -> b p n c", p=P)
    ov = o_ext.rearrange("b c h w -> b c (h w)")

    with tile.TileContext(nc) as tc, ExitStack() as ctx:
        qk_pool = ctx.enter_context(tc.tile_pool(name="qk", bufs=5))
        vb_pool = ctx.enter_context(tc.tile_pool(name="vb", bufs=6))
        vt_pool = ctx.enter_context(tc.tile_pool(name="vt", bufs=8))
        a_pool = ctx.enter_context(tc.tile_pool(name="a", bufs=3))
        at_pool = ctx.enter_context(tc.tile_pool(name="at", bufs=3))
        o_pool = ctx.enter_context(tc.tile_pool(name="o", bufs=6))
        stat_pool = ctx.enter_context(tc.tile_pool(name="stat", bufs=2 * B_LOC))
        singles = ctx.enter_context(tc.tile_pool(name="singles", bufs=1))
        ps_s = ctx.enter_context(tc.tile_pool(name="ps_s", bufs=3, space="PSUM"))
        ps_vt = ctx.enter_context(tc.tile_pool(name="ps_vt", bufs=3, space="PSUM"))
        ps_o = ctx.enter_context(tc.tile_pool(name="ps_o", bufs=2, space="PSUM"))

        ident = singles.tile([P, P], BF16)
        make_identity(nc, ident)

        for b in range(B_LOC):
            # ---- S = Q^T K, fp32, accumulate over hw ----
            # One PSUM tile (bank) per q-block: a bank can host only one
            # pending accumulation group at a time.
            s_ps = [ps_s.tile([P, C], F32, tag="ps_s", name=f"s_ps_{b}_{qb}")
                    for qb in range(QB)]
            for g in range(N_SGRP):
                # f32r matmuls run at full PE rate (1 cyc/row) vs fp32's
                # 4 cyc/row, with ~19-bit mantissa precision (measured rel
                # err 1.5e-4 on the logits). Bitcasting both DMA sides to
                # f32r keeps the copy cast-free so it can ride the HWDGE
                # ring (nc.sync), leaving the gpsimd SWDGE queue to V.
                q_t = qk_pool.tile([P, SG, C], F32R, tag="q")
                nc.sync.dma_start(
                    out=q_t[:],
                    in_=qv[b, :, g * SG:(g + 1) * SG, :].bitcast(F32R))
                k_t = qk_pool.tile([P, SG, C], F32R, tag="k")
                nc.sync.dma_start(
                    out=k_t[:],
                    in_=kv[b, :, g * SG:(g + 1) * SG, :].bitcast(F32R))
                for c in range(SG):
                    for qb in range(QB):
                        nc.tensor.matmul(
                            s_ps[qb][:],
                            lhsT=q_t[:, c, qb * P:(qb + 1) * P],
                            rhs=k_t[:, c, :],
                            start=(g == 0 and c == 0),
                            stop=(g == N_SGRP - 1 and c == SG - 1),
                        )

            # ---- softmax over k (free axis) ----
            negmax = stat_pool.tile([P, QB, 1], F32, tag="negmax")
            rowsum = stat_pool.tile([P, QB, 1], F32, tag="rowsum")
            recip = stat_pool.tile([P, QB, 1], F32, tag="recip")
            a_sb = a_pool.tile([P, QB, C], BF16, tag="a")
            for qb in range(QB):
                nc.vector.tensor_reduce(
                    out=negmax[:, qb, :], in_=s_ps[qb][:],
                    axis=mybir.AxisListType.X, op=mybir.AluOpType.max,
                    negate=True)
                nc.scalar.activation(
                    out=a_sb[:, qb, :], in_=s_ps[qb][:],
                    func=mybir.ActivationFunctionType.Exp,
                    bias=negmax[:, qb, :], scale=1.0,
                    accum_out=rowsum[:, qb, :])
                nc.vector.reciprocal(out=recip[:, qb, :], in_=rowsum[:, qb, :])

            # ---- A^T via PE transposes: at[:, kc, qb, :] = A[qb-block, kc-chunk]^T
            at_ps = ps_s.tile([P, KC, QB, P], BF16, tag="ps_s")
            for kc in range(KC):
                for qb in range(QB):
                    nc.tensor.transpose(
                        out=at_ps[:, kc, qb, :],
                        in_=a_sb[:, qb, kc * P:(kc + 1) * P],
                        identity=ident[:])
            at_sb = at_pool.tile([P, KC, QB, P], BF16, tag="at")
            nc.vector.tensor_copy(out=at_sb[:], in_=at_ps[:])

            # ---- O = A @ V^T, bf16, streamed over hw groups ----
            for g in range(N_VGRP):
                # SWDGE DMA casts f32 -> bf16 inline.
                vb_t = vb_pool.tile([P, VG, C], BF16, tag="vb")
                nc.gpsimd.dma_start(out=vb_t[:], in_=vv[b, :, g * VG:(g + 1) * VG, :])
                vt_ps = ps_vt.tile([P, KC, VG, P], BF16, tag="ps_vt")
                for c in range(VG):
                    for kc in range(KC):
                        nc.tensor.transpose(
                            out=vt_ps[:, kc, c, :],
                            in_=vb_t[:, c, kc * P:(kc + 1) * P],
                            identity=ident[:])
                vt_sb = vt_pool.tile([P, KC, VG, P], BF16, tag="vt")
                nc.vector.tensor_copy(out=vt_sb[:], in_=vt_ps[:])
                for qb in range(QB):
                    o_ps = ps_o.tile([P, VG * P], F32, tag="ps_o")
                    for kc in range(KC):
                        nc.tensor.matmul(
                            o_ps[:],
                            lhsT=at_sb[:, kc, qb, :],
                            rhs=vt_sb[:, kc, :, :].rearrange("p c x -> p (c x)"),
                            start=(kc == 0), stop=(kc == KC - 1),
                        )
                    o_sb = o_pool.tile([P, VG * P], F32, tag="o")
                    # Split epilogues between ACT and DVE to balance load.
                    if qb == 0:
                        nc.scalar.activation(
                            out=o_sb[:], in_=o_ps[:],
                            func=mybir.ActivationFunctionType.Copy,
                            scale=recip[:, qb, :])
                    else:
                        nc.vector.tensor_scalar_mul(
                            o_sb[:], o_ps[:], recip[:, qb, :])
                    nc.scalar.dma_start(
                        out=ov[b, qb * P:(qb + 1) * P,
                               g * VG * P:(g + 1) * VG * P],
                        in_=o_sb[:])

    nc.compile()
    return nc


def _get_nc():
    if "nc" not in _CACHE:
        _CACHE["nc"] = _build()
    return _CACHE["nc"]


def kernel(query, keys, values):
    query = np.ascontiguousarray(np.asarray(query, dtype=np.float32))
    keys = np.ascontiguousarray(np.asarray(keys, dtype=np.float32))
    values = np.ascontiguousarray(np.asarray(values, dtype=np.float32))
    assert query.shape == (B, H, W, C), query.shape

    nc = _get_nc()
    in_maps = []
    for i in range(N_CORES):
        sl = slice(i * B_LOC, (i + 1) * B_LOC)
        in_maps.append({
            "query": query[sl],
            "keys": keys[sl],
            "values": values[sl],
        })
    res = run_bass_kernel_spmd(nc, in_maps, core_ids=list(range(N_CORES)))
    out = np.concatenate([res.results[i]["out"] for i in range(N_CORES)], axis=0)
    return out


# revision 15
# speedup vs baseline: 1.0133x; 1.0133x over previous
"""Data-parallel attention kernel for Trainium2 (8 NeuronCores).

Reference computation (per batch item b):
    scores[q, k] = sum_{hw} query[b, hw, q] * keys[b, hw, k]     (C=256, HW=4096)
    attn = softmax_k(scores)
    out[b, q, hw] = sum_k attn[q, k] * values[b, hw, k]

Sharding: batch axis (B=32) split across 8 cores, 4 items per core, no
cross-core communication.

Per-core per-item plan:
  S phase:  fp32 matmuls, contraction over hw streamed in 4 groups of 1024
            rows, accumulating into PSUM ([128, 2, 256] = 2 q-blocks).
  softmax:  DVE row-max (negated) -> ACT exp(in + bias) with accumulated row
            sums -> DVE reciprocal. Normalization is folded into the O-phase
            PSUM->SBUF epilogue, so A stays unnormalized bf16.
  O phase:  V streamed in 8 groups of 512 rows, cast to bf16 (ACT), PE
            transposed ([hw,k] -> [k,hw]) via identity matmuls, then bf16
            matmuls A @ V^T accumulated over the 2 k-chunks; epilogue scales
            rows by 1/rowsum during the PSUM->SBUF copy and DMAs out.
"""

import numpy as np

import concourse.bass as bass
import concourse.tile as tile
from concourse import bacc, mybir
from concourse.bass_utils import run_bass_kernel_spmd
from concourse.masks import make_identity
from contextlib import ExitStack

B, H, W, C = 32, 64, 64, 256
N_CORES = 8
B_LOC = B // N_CORES          # 4 batch items per core
HW = H * W                    # 4096
P = 128                       # partitions
N_CHUNK = HW // P             # 32 chunks of 128 hw-rows
SG = 8                        # chunks per S-phase group (1024 hw rows)
VG = 4                        # chunks per O-phase group (512 hw rows)
N_SGRP = N_CHUNK // SG        # 4
N_VGRP = N_CHUNK // VG        # 8
QB = C // P                   # 2 q-blocks
KC = C // P                   # 2 k-chunks

F32 = mybir.dt.float32
F32R = mybir.dt.float32r
BF16 = mybir.dt.bfloat16

_CACHE = {}


def _build():
    nc = bacc.Bacc("TRN2", target_bir_lowering=False, debug=False,
                   num_devices=N_CORES)
    q_ext = nc.dram_tensor("query", [B_LOC, H, W, C], F32,
                           kind="ExternalInput").ap()
    k_ext = nc.dram_tensor("keys", [B_LOC, H, W, C], F32,
                           kind="ExternalInput").ap()
    v_ext = nc.dram_tensor("values", [B_LOC, H, W, C], F32,
                           kind="ExternalInput").ap()
    o_ext = nc.dram_tensor("out", [B_LOC, C, H, W], F32,
                           kind="ExternalOutput").ap()

    # [b, hw, c] -> [b, p, n, c] where hw = n*128 + p
    qv = q_ext.rearrange("b h w c -> b (h w) c").rearrange(
        "b (n p) c -> b p n c", p=P)
    kv = k_ext.rearrange("b h w c -> b (h w) c").rearrange(
        "b (n p) c -> b p n c", p=P)
    vv = v_ext.rearrange("b h w c -> b (h w) c").rearrange(
        "b (n p) c -> b p n c", p=P)
    ov = o_ext.rearrange("b c h w -> b c (h w)")

    with tile.TileContext(nc) as tc, ExitStack() as ctx:
        qk_pool = ctx.enter_context(tc.tile_pool(name="qk", bufs=5))
        vb_pool = ctx.enter_context(tc.tile_pool(name="vb", bufs=6))
        vt_pool = ctx.enter_context(tc.tile_pool(name="vt", bufs=8))
        a_pool = ctx.enter_context(tc.tile_pool(name="a", bufs=3))
        at_pool = ctx.enter_context(tc.tile_pool(name="at", bufs=3))
        o_pool = ctx.enter_context(tc.tile_pool(name="o", bufs=6))
        stat_pool = ctx.enter_context(tc.tile_pool(name="stat", bufs=2 * B_LOC))
        singles = ctx.enter_context(tc.tile_pool(name="singles", bufs=1))
        ps_s = ctx.enter_context(tc.tile_pool(name="ps_s", bufs=3, space="PSUM"))
        ps_vt = ctx.enter_context(tc.tile_pool(name="ps_vt", bufs=3, space="PSUM"))
        ps_o = ctx.enter_context(tc.tile_pool(name="ps_o", bufs=2, space="PSUM"))

        ident = singles.tile([P, P], BF16)
        make_identity(nc, ident)

        for b in range(B_LOC):
            # ---- S = Q^T K, fp32, accumulate over hw ----
            # One PSUM tile (bank) per q-block: a bank can host only one
            # pending accumulation group at a time.
            s_ps = [ps_s.tile([P, C], F32, tag="ps_s", name=f"s_ps_{b}_{qb}")
                    for qb in range(QB)]
            for g in range(N_SGRP):
                # f32r matmuls run at full PE rate (1 cyc/row) vs fp32's
                # 4 cyc/row, with ~19-bit mantissa precision (measured rel
                # err 1.5e-4 on the logits). Bitcasting both DMA sides to
                # f32r keeps the copy cast-free so it can ride the HWDGE
                # ring (nc.sync), leaving the gpsimd SWDGE queue to V.
                q_t = qk_pool.tile([P, SG, C], F32R, tag="q")
                nc.sync.dma_start(
                    out=q_t[:],
                    in_=qv[b, :, g * SG:(g + 1) * SG, :].bitcast(F32R))
                k_t = qk_pool.tile([P, SG, C], F32R, tag="k")
                nc.sync.dma_start(
                    out=k_t[:],
                    in_=kv[b, :, g * SG:(g + 1) * SG, :].bitcast(F32R))
                for c in range(SG):
                    for qb in range(QB):
                        nc.tensor.matmul(
                            s_ps[qb][:],
                            lhsT=q_t[:, c, qb * P:(qb + 1) * P],
                            rhs=k_t[:, c, :],
                            start=(g == 0 and c == 0),
                            stop=(g == N_SGRP - 1 and c == SG - 1),
                        )

            # ---- softmax over k (free axis) ----
            negmax = stat_pool.tile([P, QB, 1], F32, tag="negmax")
            rowsum = stat_pool.tile([P, QB, 1], F32, tag="rowsum")
            recip = stat_pool.tile([P, QB, 1], F32, tag="recip")
            a_sb = a_pool.tile([P, QB, C], BF16, tag="a")
            for qb in range(QB):
                nc.vector.tensor_reduce(
                    out=negmax[:, qb, :], in_=s_ps[qb][:],
                    axis=mybir.AxisListType.X, op=mybir.AluOpType.max,
                    negate=True)
                nc.scalar.activation(
                    out=a_sb[:, qb, :], in_=s_ps[qb][:],
                    func=mybir.ActivationFunctionType.Exp,
                    bias=negmax[:, qb, :], scale=1.0,
                    accum_out=rowsum[:, qb, :])
                nc.vector.reciprocal(out=recip[:, qb, :], in_=rowsum[:, qb, :])

            # ---- A^T via PE transposes: at[:, kc, qb, :] = A[qb-block, kc-chunk]^T
            at_ps = ps_s.tile([P, KC, QB, P], BF16, tag="ps_s")
            for kc in range(KC):
                for qb in range(QB):
                    nc.tensor.transpose(
                        out=at_ps[:, kc, qb, :],
                        in_=a_sb[:, qb, kc * P:(kc + 1) * P],
                        identity=ident[:])
            at_sb = at_pool.tile([P, KC, QB, P], BF16, tag="at")
            nc.vector.tensor_copy(out=at_sb[:], in_=at_ps[:])

            # ---- O = A @ V^T, bf16, streamed over hw groups ----
            for g in range(N_VGRP):
                # SWDGE DMA casts f32 -> bf16 inline.
                vb_t = vb_pool.tile([P, VG, C], BF16, tag="vb")
                nc.gpsimd.dma_start(out=vb_t[:], in_=vv[b, :, g * VG:(g + 1) * VG, :])
                vt_ps = ps_vt.tile([P, KC, VG, P], BF16, tag="ps_vt")
                for c in range(VG):
                    for kc in range(KC):
                        nc.tensor.transpose(
                            out=vt_ps[:, kc, c, :],
                            in_=vb_t[:, c, kc * P:(kc + 1) * P],
                            identity=ident[:])
                vt_sb = vt_pool.tile([P, KC, VG, P], BF16, tag="vt")
                nc.vector.tensor_copy(out=vt_sb[:], in_=vt_ps[:])
                for qb in range(QB):
                    o_ps = ps_o.tile([P, VG * P], F32, tag="ps_o")
                    for kc in range(KC):
                        nc.tensor.matmul(
                            o_ps[:],
                            lhsT=at_sb[:, kc, qb, :],
                            rhs=vt_sb[:, kc, :, :].rearrange("p c x -> p (c x)"),
                            start=(kc == 0), stop=(kc == KC - 1),
                        )
                    o_sb = o_pool.tile([P, VG * P], F32, tag="o")
                    # Split epilogues between ACT and DVE to balance load.
                    if qb == 0:
                        nc.scalar.activation(
                            out=o_sb[:], in_=o_ps[:],
                            func=mybir.ActivationFunctionType.Copy,
                            scale=recip[:, qb, :])
                    else:
                        nc.vector.tensor_scalar_mul(
                            o_sb[:], o_ps[:], recip[:, qb, :])
                    nc.sync.dma_start(
                        out=ov[b, qb * P:(qb + 1) * P,
                               g * VG * P:(g + 1) * VG * P],
                        in_=o_sb[:])

    nc.compile()
    return nc


def _get_nc():
    if "nc" not in _CACHE:
        _CACHE["nc"] = _build()
    return _CACHE["nc"]


def kernel(query, keys, values):
    query = np.ascontiguousarray(np.asarray(query, dtype=np.float32))
    keys = np.ascontiguousarray(np.asarray(keys, dtype=np.float32))
    values = np.ascontiguousarray(np.asarray(values, dtype=np.float32))
    assert query.shape == (B, H, W, C), query.shape

    nc = _get_nc()
    in_maps = []
    for i in range(N_CORES):
        sl = slice(i * B_LOC, (i + 1) * B_LOC)
        in_maps.append({
            "query": query[sl],
            "keys": keys[sl],
            "values": values[sl],
        })
    res = run_bass_kernel_spmd(nc, in_maps, core_ids=list(range(N_CORES)))
    out = np.concatenate([res.results[i]["out"] for i in range(N_CORES)], axis=0)
    return out
